# revision 23
# baseline (speedup 1.0000x reference)
"""BlockNTP transformer forward on 8 trn2 cores.

Sharding:
  - cores 0-3 -> batch 0, cores 4-7 -> batch 1 (group of 4 per batch elem)
  - within a group, core position g owns 512 of the 2048 sequence rows:
    real-token rows [256g, 256(g+1)) and mask-token rows [256g, 256(g+1)),
    stored locally as [real 256 | mask 256].
  - transformer blocks run sequence-sharded with one KV AllGather per layer
    (replica groups {0..3}, {4..7}).
  - tied projection + CE partials are vocab-sharded: core c owns vocab
    [4000c, 4000(c+1)) and computes logits for ALL 4096 rows over its shard.
Host does: embedding lookup (input prep), vocab/row reassembly, final CE
combine from per-core sum-exp partials.
"""
import sys
sys.path.insert(0, '/opt/trn_rl_repo')

import os
import numpy as np
import ml_dtypes

import concourse.bass as bass
import concourse.tile as tile
import concourse.mybir as mybir
from concourse import bacc
from concourse import bass_utils

F32 = mybir.dt.float32
F32R = mybir.dt.float32r
BF16 = mybir.dt.bfloat16

P = 128
B, T, L = 2, 1024, 16
D, H, DFF, V = 512, 8, 2048, 32000
NB, ND = 4, 2
NL = NB + ND
DH = D // H
S = 2 * T                 # 2048 rows per batch elem
DC = D // P               # 4 d-chunks
RT = 4                    # local row tiles (512 rows)
KT = 16                   # global k tiles
FT = DFF // P             # 16
N_CORES = 8
VS = V // N_CORES         # 4000 vocab per core
NVT = 8                   # vocab tiles per core
VT = VS // NVT            # 500
GRT = 32                  # global row tiles (4096 rows)
EPS = 1e-5

_CACHE = {}


def _bcast_rows(ap, p=P):
    """Prepend a 0-stride partition dim: [..] -> [p, ..] broadcast DMA src."""
    return bass.AP(tensor=ap.tensor, offset=ap.offset,
                   ap=[[0, p]] + [list(x) for x in ap.ap])


def _out_row_base(rt_ag):
    """AG row-tile index -> natural (b, seq) row base in the 4096-row output."""
    r, lt = rt_ag // 4, rt_ag % 4
    b, g = r // 4, r % 4
    if lt < 2:
        seq = 128 * (2 * g + lt)
    else:
        seq = T + 128 * (2 * g + (lt - 2))
    return 2048 * b + seq


CE_RTS = [rt for rt in range(GRT) if rt % 4 >= 2]   # mask-row tiles (seq>=1024)


DBG_LAYERS = int(os.environ.get("DBG_LAYERS", str(NL)))
DBG_PROJ = int(os.environ.get("DBG_PROJ", "1"))
DBG_PHASE = int(os.environ.get("DBG_PHASE", "99"))
DBG_ATT = int(os.environ.get("DBG_ATT", "99"))


def build_program(trivial_ln, trivial_bias):
    nc = bacc.Bacc("TRN2", target_bir_lowering=False, debug=False,
                   enable_asserts=False, num_devices=N_CORES)

    # ---------------- DRAM I/O ----------------
    x0_d = nc.dram_tensor("x0", [512, D], F32, kind="ExternalInput")
    wqkv_d = nc.dram_tensor("wqkv", [NL, D, 3 * D], F32R, kind="ExternalInput")
    wo_d = nc.dram_tensor("wo", [NL, H, DH, D], F32R, kind="ExternalInput")
    w1_d = nc.dram_tensor("w1", [NL, D, DFF], F32R, kind="ExternalInput")
    w2_d = nc.dram_tensor("w2", [NL, DFF, D], BF16, kind="ExternalInput")
    bqkv_d = nc.dram_tensor("bqkv", [NL, 3 * D], F32, kind="ExternalInput")
    maskr_d = nc.dram_tensor("mask_r", [2, 8, P, 512], BF16, kind="ExternalInput")
    maskm_d = nc.dram_tensor("mask_m", [2, 8, P, 256], BF16, kind="ExternalInput")
    eT_d = nc.dram_tensor("eT", [DC, P, VS], F32R, kind="ExternalInput")
    ident_d = nc.dram_tensor("ident", [P, P], F32R, kind="ExternalInput")
    ones1_d = nc.dram_tensor("ones1", [P, 64], F32R, kind="ExternalInput")
    if not trivial_ln:
        ln1s_d = nc.dram_tensor("ln1s", [NL, D], F32, kind="ExternalInput")
        ln1b_d = nc.dram_tensor("ln1b", [NL, D], F32, kind="ExternalInput")
        ln2s_d = nc.dram_tensor("ln2s", [NL, D], F32, kind="ExternalInput")
        ln2b_d = nc.dram_tensor("ln2b", [NL, D], F32, kind="ExternalInput")
        lnfs_d = nc.dram_tensor("lnfs", [D], F32, kind="ExternalInput")
        lnfb_d = nc.dram_tensor("lnfb", [D], F32, kind="ExternalInput")
    if not trivial_bias:
        bo_d = nc.dram_tensor("bo", [NL, D], F32, kind="ExternalInput")
        b1_d = nc.dram_tensor("b1", [NL, DFF], F32, kind="ExternalInput")
        b2_d = nc.dram_tensor("b2", [NL, D], F32, kind="ExternalInput")

    logits_d = nc.dram_tensor("logits", [4096, VS], F32, kind="ExternalOutput")
    ces_d = nc.dram_tensor("ce_s", [len(CE_RTS), P, NVT], F32, kind="ExternalOutput")

    with tile.TileContext(nc) as tc:
        with tc.tile_pool(name="const", bufs=1) as const, \
             tc.tile_pool(name="persist", bufs=1) as persist, \
             tc.tile_pool(name="big", bufs=1) as big, \
             tc.tile_pool(name="med", bufs=1) as med, \
             tc.tile_pool(name="wpan", bufs=1) as wpan, \
             tc.tile_pool(name="wts", bufs=3) as wts, \
             tc.tile_pool(name="sm", bufs=1) as sm, \
             tc.tile_pool(name="exps", bufs=3) as exps, \
             tc.tile_pool(name="proj", bufs=2) as proj, \
             tc.tile_pool(name="lnp", bufs=2) as lnp, \
             tc.tile_pool(name="psA", bufs=2, space="PSUM") as psA, \
             tc.tile_pool(name="psAV", bufs=2, space="PSUM") as psAV, \
             tc.tile_pool(name="dram", bufs=1, space="DRAM") as dram:

            # ---- constants ----
            ident = const.tile([P, P], F32R, tag="ident")
            nc.sync.dma_start(ident[:], ident_d[:])
            ones1 = const.tile([P, 64], F32R, tag="ones1")
            nc.sync.dma_start(ones1[:], ones1_d[:])
            eps_sb = const.tile([P, 1], F32, tag="eps")
            nc.vector.memset(eps_sb[:], EPS)

            # ---- x state (persistent across layers) ----
            x_cur = persist.tile([P, RT, D], F32, tag="x_cur")
            nc.sync.dma_start(
                x_cur[:], x0_d[:].rearrange("(rt p) d -> p rt d", p=P))

            def layer_norm(src_tile, h_tile, s_ap, b_ap):
                """h = LN(src) rowwise over D; h_tile dtype f32r."""
                if not trivial_ln:
                    s_bc = lnp.tile([P, D], F32, tag="ln_s")
                    b_bc = lnp.tile([P, D], F32, tag="ln_b")
                    nc.sync.dma_start(s_bc[:], _bcast_rows(s_ap))
                    nc.sync.dma_start(b_bc[:], _bcast_rows(b_ap))
                for rt in range(RT):
                    stats = lnp.tile([P, 6], F32, tag="ln_stats")
                    mv = lnp.tile([P, 2], F32, tag="ln_mv")
                    nc.vector.bn_stats(out=stats[:], in_=src_tile[:, rt, :])
                    nc.vector.bn_aggr(out=mv[:], in_=stats[:])
                    sd = lnp.tile([P, 2], F32, tag="ln_sd")
                    nc.scalar.activation(out=sd[:, 0:1], in_=mv[:, 1:2],
                                         func=mybir.ActivationFunctionType.Sqrt,
                                         bias=eps_sb[:], scale=1.0)
                    nc.vector.reciprocal(out=sd[:, 1:2], in_=sd[:, 0:1])
                    if trivial_ln:
                        nc.vector.tensor_scalar(
                            out=h_tile[:, rt, :], in0=src_tile[:, rt, :],
                            scalar1=mv[:, 0:1], scalar2=sd[:, 1:2],
                            op0=mybir.AluOpType.subtract,
                            op1=mybir.AluOpType.mult)
                    else:
                        tmp = lnp.tile([P, D], F32, tag="ln_tmp")
                        nc.vector.tensor_scalar(
                            out=tmp[:], in0=src_tile[:, rt, :],
                            scalar1=mv[:, 0:1], scalar2=sd[:, 1:2],
                            op0=mybir.AluOpType.subtract,
                            op1=mybir.AluOpType.mult)
                        nc.vector.tensor_mul(out=tmp[:], in0=tmp[:], in1=s_bc[:])
                        nc.vector.tensor_add(out=h_tile[:, rt, :], in0=tmp[:],
                                             in1=b_bc[:])

            def transpose_to(h_tile, hT_tile):
                """[P, RT, D] rows-major -> [P, DC, 512] d-major (f32r)."""
                for dc in range(DC):
                    for rt in range(RT):
                        ptp = psA.tile([P, 1024], F32R, tag="A")
                        nc.tensor.transpose(
                            out=ptp[:, :P], in_=h_tile[:, rt, bass.ts(dc, P)],
                            identity=ident[:])
                        nc.any.tensor_copy(
                            out=hT_tile[:, dc, bass.ts(rt, P)], in_=ptp[:, :P])

            for l in range(DBG_LAYERS):
                st = 0 if l < NB else 1

                # ---------- LN1 + transpose ----------
                h_sb = med.tile([P, RT, D], F32R, tag="h")
                layer_norm(x_cur, h_sb,
                           None if trivial_ln else ln1s_d[l],
                           None if trivial_ln else ln1b_d[l])
                hT_sb = med.tile([P, DC, 512], F32R, tag="hT")
                transpose_to(h_sb, hT_sb)

                # ---------- QKV ----------
                qkvb = lnp.tile([P, 12], F32, tag="qkvb")
                nc.sync.dma_start(
                    qkvb[:], bqkv_d[l].rearrange("(t p) -> p t", p=P))
                # Q_T / K_T  (swapped: out [c, r])
                qkT = med.tile([P, 8, 512], F32R, tag="qkT")
                for ct in range(8):
                    wq = wts.tile([P, DC, P], F32R, tag="w_kx128")
                    nc.sync.dma_start(
                        wq[:], wqkv_d[l, :, bass.ts(ct, P)].rearrange(
                            "(dc p) c -> p dc c", p=P))
                    pq = psA.tile([P, 1024], F32, tag="A")
                    for dc in range(DC):
                        nc.tensor.matmul(pq[:, :512], lhsT=wq[:, dc, :],
                                         rhs=hT_sb[:, dc, :],
                                         start=(dc == 0), stop=(dc == DC - 1))
                    nc.vector.tensor_scalar_add(
                        out=qkT[:, ct, :], in0=pq[:, :512],
                        scalar1=qkvb[:, ct:ct + 1])
                # V natural (out [r, c]) -> bf16
                wv = med.tile([P, DC, 512], F32R, tag="wv_attnT")
                nc.sync.dma_start(
                    wv[:], wqkv_d[l, :, 1024:].rearrange("(dc p) c -> p dc c", p=P))
                v_sb = med.tile([P, RT, 512], BF16, tag="v")
                if not trivial_bias:
                    bv_bc = lnp.tile([P, 512], F32, tag="bv")
                    nc.sync.dma_start(bv_bc[:], _bcast_rows(bqkv_d[l, 1024:]))
                for rt in range(RT):
                    pv = psA.tile([P, 1024], F32, tag="A")
                    for dc in range(DC):
                        nc.tensor.matmul(pv[:, :512],
                                         lhsT=hT_sb[:, dc, bass.ts(rt, P)],
                                         rhs=wv[:, dc, :],
                                         start=(dc == 0), stop=(dc == DC - 1))
                    if trivial_bias:
                        nc.any.tensor_copy(out=v_sb[:, rt, :], in_=pv[:, :512])
                    else:
                        nc.vector.tensor_add(out=v_sb[:, rt, :],
                                             in0=pv[:, :512], in1=bv_bc[:])

                if DBG_PHASE <= 1:
                    nc.sync.dma_start(logits_d[0:P, 0:3584],
                                      qkT[:, :7, :].bitcast(F32))
                    nc.sync.dma_start(logits_d[P:2*P, 0:512],
                                      v_sb[:, :, :].rearrange("p a b -> p (a b)")[:, :1024].bitcast(F32))
                    continue
                # ---------- KV export + AllGather ----------
                kv_loc = dram.tile([P, 4, 1536], BF16, tag="kv_loc")
                nc.sync.dma_start(
                    kv_loc[:, :, :1024].bitcast(F32R), qkT[:, 4:8, :])
                nc.sync.dma_start(kv_loc[:, :, 1024:], v_sb[:])
                kv_glob = dram.tile([4, P, 4, 1536], BF16, tag="kv_glob")
                nc.gpsimd.collective_compute(
                    "AllGather", mybir.AluOpType.bypass,
                    replica_groups=[[0, 1, 2, 3], [4, 5, 6, 7]],
                    ins=[kv_loc[:].opt()], outs=[kv_glob[:].opt()])

                # ---------- import K_T full + Vones ----------
                kT_full = big.tile([P, DC, S], F32R, tag="bigA")
                vones = big.tile([P, KT, 520], BF16, tag="bigB")
                vview = vones[:].rearrange("p t (h e) -> p t h e", e=65)
                for r in range(4):
                    nc.sync.dma_start(
                        kT_full[:, :, bass.ds(512 * r, 512)],
                        kv_glob[r, :, :, :1024].bitcast(F32R))
                    for lt in range(4):
                        srcv = kv_glob[r, :, lt, 1024:].rearrange(
                            "p (h e) -> p h e", e=64)
                        nc.sync.dma_start(
                            vview[:, 4 * r + lt, :, 0:64], srcv[:])
                nc.vector.memset(vview[:, :, :, 64:65], 1.0)

                # ---------- masks for this layer's set ----------
                mr_sb = sm.tile([P, 8, 512], BF16, tag="mask_r")
                mm_sb = sm.tile([P, 8, 256], BF16, tag="mask_m")
                nc.sync.dma_start(
                    mr_sb[:], maskr_d[st].rearrange("j p q -> p j q"))
                nc.sync.dma_start(
                    mm_sb[:], maskm_d[st].rearrange("j p q -> p j q"))

                if DBG_PHASE <= 2:
                    nc.sync.dma_start(logits_d[0:P, 0:2048],
                                      kT_full[:, 0, :].bitcast(F32))
                    nc.sync.dma_start(logits_d[P:2*P, 0:1040],
                                      vones[:, :4, :].rearrange("p a b -> p (a b)").bitcast(F32))
                    continue
                # ---------- attention ----------
                attnT = med.tile([64, 8, 512], F32R, tag="wv_attnT")
                for hg in range(4):          # 2 heads per group
                    heads = (2 * hg, 2 * hg + 1)
                    pav = ([psAV.tile([P, 512], F32, tag="av", name=f"pav{hg}_{i}")
                            for i in range(2)] if DBG_ATT >= 3 else [None, None])
                    pavm = ([psAV.tile([P, 256], F32, tag="avm", name=f"pavm{hg}_{i}")
                             for i in range(2)] if DBG_ATT >= 3 else [None, None])
                    for kt in range(KT):
                        ltk = kt % 4
                        is_real = ltk < 2
                        qlo, qn = (0, 512) if is_real else (256, 256)
                        ps = psA.tile([P, 1024], F32, tag="A")
                        es = exps.tile([P, 2, 512], BF16, tag="expS")
                        for hi, h in enumerate(heads):
                            po = 64 * (h % 2)
                            cth = h // 2
                            nc.tensor.matmul(
                                ps[:, bass.ds(hi * 512, qn)],
                                lhsT=kT_full[po:po + 64, cth, bass.ts(kt, P)],
                                rhs=qkT[po:po + 64, cth, bass.ds(qlo, qn)],
                                start=True, stop=True)
                        nc.scalar.activation(
                            out=es[:, :, :qn],
                            in_=ps[:].rearrange("p (a b) -> p a b", b=512)[:, :, :qn],
                            func=mybir.ActivationFunctionType.Exp,
                            bias=0.0, scale=0.125)
                        if is_real:
                            msk = mr_sb[:, 2 * (kt // 4) + ltk, :]
                        else:
                            msk = mm_sb[:, 2 * (kt // 4) + (ltk - 2), :]
                        if DBG_ATT >= 2:
                            nc.vector.tensor_tensor(
                                out=es[:, :, :qn], in0=es[:, :, :qn],
                                in1=msk.unsqueeze(1).to_broadcast([P, 2, qn]),
                                op=mybir.AluOpType.mult)
                        if DBG_ATT >= 3:
                            for hi, h in enumerate(heads):
                                if is_real:
                                    nc.tensor.matmul(
                                        pav[hi][0:65, 0:512],
                                        lhsT=vones[:, kt, bass.ds(65 * h, 65)],
                                        rhs=es[:, hi, :qn],
                                        start=(kt == 0), stop=(kt == KT - 2))
                                else:
                                    nc.tensor.matmul(
                                        pavm[hi][0:65, 0:256],
                                        lhsT=vones[:, kt, bass.ds(65 * h, 65)],
                                        rhs=es[:, hi, :qn],
                                        start=(kt == 2), stop=(kt == KT - 1))
                        if DBG_ATT < 3:
                            nc.gpsimd.dma_start(
                                logits_d[0:P, bass.ds(512 * (kt % 4), 512)],
                                es[:, 0, :])
                    # normalize + write attn_T (all heads at partitions 0-63)
                    for hi, h in enumerate(heads):
                        if DBG_ATT < 3:
                            continue
                        if DBG_ATT < 4:
                            nc.any.tensor_copy(out=attnT[:, h, :],
                                               in_=pav[hi][0:64, :])
                            continue
                        cs = lnp.tile([P, 512], F32, tag="cs")
                        nc.any.tensor_copy(out=cs[0:65, :], in_=pav[hi][0:65, :])
                        nc.vector.tensor_add(out=cs[0:65, 256:512],
                                             in0=cs[0:65, 256:512],
                                             in1=pavm[hi][0:65, :])
                        rden = lnp.tile([P, 512], F32R, tag="rden")
                        with nc.allow_low_precision(reason="f32r denom bcast"):
                            nc.vector.reciprocal(out=rden[64:65, :],
                                                 in_=cs[64:65, :])
                        pb = psA.tile([P, 1024], F32, tag="A")
                        nc.tensor.matmul(pb[0:64, :512],
                                         lhsT=ones1[64:65, :],
                                         rhs=rden[64:65, :],
                                         start=True, stop=True)
                        bc = lnp.tile([P, 512], F32, tag="bcast")
                        nc.any.tensor_copy(out=bc[0:64, :], in_=pb[0:64, :512])
                        nc.vector.tensor_tensor(
                            out=attnT[:, h, :],
                            in0=cs[0:64, :], in1=bc[0:64, :],
                            op=mybir.AluOpType.mult)

                if DBG_PHASE <= 3:
                    if DBG_ATT >= 3:
                        nc.sync.dma_start(logits_d[0:64, 0:4000],
                                          attnT[:].rearrange("p a b -> p (a b)")[:, :4000].bitcast(F32))
                    continue
                # ---------- Wo + residual ----------
                wo_sb = wpan.tile([64, H, 512], F32R, tag="wo")
                nc.sync.dma_start(
                    wo_sb[:], wo_d[l].rearrange("h p d -> p h d"))
                if not trivial_bias:
                    bo_bc = lnp.tile([P, 512], F32, tag="bo")
                    nc.sync.dma_start(bo_bc[:], _bcast_rows(bo_d[l]))
                for rt in range(RT):
                    pw = psA.tile([P, 1024], F32, tag="A")
                    for h in range(H):
                        nc.tensor.matmul(pw[:, :512],
                                         lhsT=attnT[:, h, bass.ts(rt, P)],
                                         rhs=wo_sb[:, h, :],
                                         start=(h == 0), stop=(h == H - 1))
                    nc.vector.tensor_add(out=x_cur[:, rt, :],
                                         in0=x_cur[:, rt, :], in1=pw[:, :512])
                    if not trivial_bias:
                        nc.vector.tensor_add(out=x_cur[:, rt, :],
                                             in0=x_cur[:, rt, :], in1=bo_bc[:])

                if DBG_PHASE <= 4:
                    continue
                # ---------- LN2 + transpose ----------
                h2_sb = med.tile([P, RT, D], F32R, tag="h")
                layer_norm(x_cur, h2_sb,
                           None if trivial_ln else ln2s_d[l],
                           None if trivial_ln else ln2b_d[l])
                h2T = med.tile([P, DC, 512], F32R, tag="hT")
                transpose_to(h2_sb, h2T)

                # ---------- FFN ----------
                b1col = lnp.tile([P, FT], F32, tag="b1col")
                if not trivial_bias:
                    nc.sync.dma_start(
                        b1col[:], b1_d[l].rearrange("(t p) -> p t", p=P))
                gT = med.tile([P, FT, 512], BF16, tag="qkT")
                for ft in range(FT):
                    w1t = wts.tile([P, DC, P], F32R, tag="w_kx128")
                    nc.sync.dma_start(
                        w1t[:], w1_d[l, :, bass.ts(ft, P)].rearrange(
                            "(dc p) c -> p dc c", p=P))
                    pu = psA.tile([P, 1024], F32, tag="A")
                    for dc in range(DC):
                        nc.tensor.matmul(pu[:, :512], lhsT=w1t[:, dc, :],
                                         rhs=h2T[:, dc, :],
                                         start=(dc == 0), stop=(dc == DC - 1))
                    nc.scalar.activation(
                        out=gT[:, ft, :], in_=pu[:, :512],
                        func=mybir.ActivationFunctionType.Gelu_apprx_tanh,
                        bias=(0.0 if trivial_bias else b1col[:, ft:ft + 1]),
                        scale=1.0)
                w2_sb = wpan.tile([P, FT, 512], BF16, tag="w2")
                nc.sync.dma_start(
                    w2_sb[:], w2_d[l].rearrange("(ft p) d -> p ft d", p=P))
                if not trivial_bias:
                    b2_bc = lnp.tile([P, 512], F32, tag="bo")
                    nc.sync.dma_start(b2_bc[:], _bcast_rows(b2_d[l]))
                for rt in range(RT):
                    py = psA.tile([P, 1024], F32, tag="A")
                    for ft in range(FT):
                        nc.tensor.matmul(py[:, :512],
                                         lhsT=gT[:, ft, bass.ts(rt, P)],
                                         rhs=w2_sb[:, ft, :],
                                         start=(ft == 0), stop=(ft == FT - 1))
                    nc.vector.tensor_add(out=x_cur[:, rt, :],
                                         in0=x_cur[:, rt, :], in1=py[:, :512])
                    if not trivial_bias:
                        nc.vector.tensor_add(out=x_cur[:, rt, :],
                                             in0=x_cur[:, rt, :], in1=b2_bc[:])

            # ---------- final LN + x AllGather ----------
            xf = med.tile([P, RT, D], F32R, tag="h")
            layer_norm(x_cur, xf,
                       None if trivial_ln else lnfs_d[:],
                       None if trivial_ln else lnfb_d[:])
            xfT = med.tile([P, DC, 512], F32R, tag="hT")
            transpose_to(xf, xfT)
            xf_loc = dram.tile([P, DC, 512], F32R, tag="xf_loc")
            nc.sync.dma_start(xf_loc[:], xfT[:])
            xf_glob = dram.tile([N_CORES, P, DC, 512], F32R, tag="xf_glob")
            nc.gpsimd.collective_compute(
                "AllGather", mybir.AluOpType.bypass,
                replica_groups=[list(range(N_CORES))],
                ins=[xf_loc[:].opt()], outs=[xf_glob[:].opt()])

            # ---------- projection + CE ----------
            ce_i_map = {rt: i for i, rt in enumerate(CE_RTS)}
            for vg in range(2 if DBG_PROJ else 0):
                eT_sb = big.tile([P, DC, 2000], F32R, tag="bigA",
                                 name=f"eT{vg}")
                nc.sync.dma_start(
                    eT_sb[:], eT_d[:, :, bass.ds(vg * 2000, 2000)].rearrange(
                        "dc p v -> p dc v"))
                for rt_ag in range(GRT):
                    r, lt = rt_ag // 4, rt_ag % 4
                    xt = proj.tile([P, DC, P], F32R, tag="xfT_t")
                    nc.sync.dma_start(xt[:], xf_glob[r, :, :, bass.ts(lt, P)])
                    is_ce = rt_ag in ce_i_map
                    if is_ce:
                        ce_acc = proj.tile([P, 4], F32, tag="ce_acc")
                    row0 = _out_row_base(rt_ag)
                    for vt in range(4):
                        pp = psA.tile([P, 1024], F32, tag="A")
                        for dc in range(DC):
                            nc.tensor.matmul(
                                pp[:, :VT], lhsT=xt[:, dc, :],
                                rhs=eT_sb[:, dc, bass.ds(vt * VT, VT)],
                                start=(dc == 0), stop=(dc == DC - 1))
                        lg = proj.tile([P, VT], F32, tag="lg")
                        nc.any.tensor_copy(out=lg[:], in_=pp[:, :VT])
                        nc.sync.dma_start(
                            logits_d[row0:row0 + P,
                                     bass.ds(vg * 2000 + vt * VT, VT)], lg[:])
                        if is_ce:
                            dump = proj.tile([P, VT], F32, tag="dump")
                            nc.scalar.activation(
                                out=dump[:], in_=lg[:],
                                func=mybir.ActivationFunctionType.Exp,
                                bias=0.0, scale=1.0,
                                accum_out=ce_acc[:, vt:vt + 1])
                    if is_ce:
                        nc.sync.dma_start(
                            ces_d[ce_i_map[rt_ag], :, bass.ds(vg * 4, 4)],
                            ce_acc[:])

    nc.compile()
    return nc



# ===================== host side =====================

def _np_masks():
    r = np.arange(S)[:, None]
    c = np.arange(S)[None, :]
    top = (r < T) & (c <= r)
    ch = (r - T) // L
    bot_b = (r >= T) & ((c < ch * L) | ((c >= T + ch * L) & (c < T + (ch + 1) * L)))
    bot_a = (r >= T) & ((c <= r - T) | (c == r))
    return top | bot_b, top | bot_a


def _prep_inputs(inputs):
    tok_ids = np.asarray(inputs['tok_ids'])
    tok_emb = np.asarray(inputs['tok_emb'], dtype=np.float32)
    pos_emb = np.asarray(inputs['pos_emb'], dtype=np.float32)
    mask_tokens = np.asarray(inputs['mask_tokens'], dtype=np.float32)

    stack = lambda n: np.ascontiguousarray(np.concatenate(
        [np.asarray(inputs['body_' + n], dtype=np.float32),
         np.asarray(inputs['dec_' + n], dtype=np.float32)], axis=0))
    wqkv = stack('Wqkv'); wo = stack('Wo'); w1 = stack('W1'); w2 = stack('W2')
    wo = np.ascontiguousarray(wo.reshape(NL, H, DH, D))
    bqkv = stack('bqkv'); bo = stack('bo'); b1 = stack('b1'); b2 = stack('b2')
    ln1s = stack('ln1_s'); ln1b = stack('ln1_b')
    ln2s = stack('ln2_s'); ln2b = stack('ln2_b')
    lnfs = np.asarray(inputs['ln_f_s'], dtype=np.float32)
    lnfb = np.asarray(inputs['ln_f_b'], dtype=np.float32)

    trivial_ln = bool(np.all(ln1s == 1) and np.all(ln1b == 0)
                      and np.all(ln2s == 1) and np.all(ln2b == 0)
                      and np.all(lnfs == 1) and np.all(lnfb == 0))
    trivial_bias = bool(np.all(bqkv == 0) and np.all(bo == 0)
                        and np.all(b1 == 0) and np.all(b2 == 0))

    mask_b, mask_a = _np_masks()
    masks = [mask_b, mask_a]

    w2_bf = w2.astype(ml_dtypes.bfloat16)
    eT = np.ascontiguousarray(tok_emb.T)          # [D, V]

    in_maps = []
    for core in range(N_CORES):
        b_idx, g = core // 4, core % 4
        rseq = np.arange(256 * g, 256 * g + 256)
        x0_real = tok_emb[tok_ids[b_idx, rseq]] + pos_emb[rseq]
        x0_mask = mask_tokens[0, rseq % L] + pos_emb[rseq]
        x0 = np.ascontiguousarray(
            np.concatenate([x0_real, x0_mask], axis=0), dtype=np.float32)

        qrows = np.concatenate([rseq, T + rseq])
        mask_r = np.zeros((2, 8, P, 512), np.float32)
        mask_m = np.zeros((2, 8, P, 256), np.float32)
        for s_i in range(2):
            M = masks[s_i]
            for kt in range(KT):
                rr, lt = kt // 4, kt % 4
                if lt < 2:
                    kbase = 256 * rr + 128 * lt
                    mask_r[s_i, 2 * rr + lt] = \
                        M[np.ix_(qrows, np.arange(kbase, kbase + P))].T
                else:
                    kbase = T + 256 * rr + 128 * (lt - 2)
                    mask_m[s_i, 2 * rr + (lt - 2)] = \
                        M[np.ix_(qrows[256:], np.arange(kbase, kbase + P))].T

        eT_shard = np.ascontiguousarray(
            eT[:, VS * core: VS * (core + 1)].reshape(DC, P, VS))

        im = {
            'x0': x0,
            'ident': np.eye(P, dtype=np.float32),
            'ones1': np.ones((P, 64), dtype=np.float32),
            'wqkv': wqkv, 'wo': wo, 'w1': w1, 'w2': w2_bf,
            'bqkv': bqkv,
            'mask_r': mask_r.astype(ml_dtypes.bfloat16),
            'mask_m': mask_m.astype(ml_dtypes.bfloat16),
            'eT': eT_shard,
        }
        if not trivial_ln:
            im.update(ln1s=ln1s, ln1b=ln1b, ln2s=ln2s, ln2b=ln2b,
                      lnfs=lnfs, lnfb=lnfb)
        if not trivial_bias:
            im.update(bo=bo, b1=b1, b2=b2)
        in_maps.append(im)
    return in_maps, trivial_ln, trivial_bias


def kernel(**inputs):
    in_maps, trivial_ln, trivial_bias = _prep_inputs(inputs)

    key = (trivial_ln, trivial_bias)
    if key not in _CACHE:
        _CACHE[key] = build_program(trivial_ln, trivial_bias)
    nc = _CACHE[key]

    needed = set()
    for alloc in nc.m.functions[0].allocations:
        if isinstance(alloc, mybir.MemoryLocationSet) and alloc.kind == "ExternalInput":
            needed.add(alloc.memorylocations[0].name)
    in_maps = [{k: v for k, v in im.items() if k in needed} for im in in_maps]

    res = bass_utils.run_bass_kernel_spmd(nc, in_maps, core_ids=list(range(N_CORES)))

    # ---- reassemble logits ----
    logits = np.empty((B * S, V), dtype=np.float32)
    for core in range(N_CORES):
        logits[:, VS * core: VS * (core + 1)] = res.results[core]['logits']
    logits = logits.reshape(B, S, V)

    # ---- loss from CE partials ----
    tok_ids = np.asarray(inputs['tok_ids'])
    sums = np.zeros((B * S,), dtype=np.float64)
    for core in range(N_CORES):
        ces = res.results[core]['ce_s']          # [16, 128, 8]
        for i, rt_ag in enumerate(CE_RTS):
            row0 = _out_row_base(rt_ag)
            sums[row0:row0 + P] += ces[i].sum(axis=1, dtype=np.float64)
    sums = sums.reshape(B, S)
    labels = np.asarray(tok_ids)[:, 1:]           # [B, T-1]
    lse = np.log(sums[:, T:S - 1])                # [B, T-1]
    lab_logit = np.take_along_axis(
        logits[:, T:S - 1, :], labels[..., None], axis=-1)[..., 0]
    loss = np.float32(np.mean(lse - lab_logit))

    return logits, loss
